# revision 1
# baseline (speedup 1.0000x reference)
"""Bass/Trainium2 kernel for framed 2-layer BiLSTM (nn_BLSTM).

Data-parallel over the 80 framed sequences: 10 per core on 8 NeuronCores.
Each core runs the full network on its shard: input projections (batched
matmuls), both LSTM directions per layer (interleaved recurrences), final
linear. Host does framing/unframing/skip-add only.
"""
import sys
import numpy as np

sys.path.insert(0, "/opt/trn_rl_repo")

import concourse.bass as bass  # noqa: E402
import concourse.mybir as mybir  # noqa: E402
from concourse import bacc  # noqa: E402
from concourse.tile import TileContext  # noqa: E402
from concourse.masks import make_identity  # noqa: E402
from concourse.bass_utils import run_bass_kernel_spmd  # noqa: E402

F32 = mybir.dt.float32
F32R = mybir.dt.float32r

DIM = 768
H = 768
G = 4 * H            # 3072, gate order reordered to [i, f, o, g]
B, T = 4, 2000
WIDTH, STRIDE = 200, 100
NFR = 20             # frames per batch element
NSEQ = B * NFR       # 80
NCORES = 8
SEQ_PC = NSEQ // NCORES   # 10
ROWS = SEQ_PC * WIDTH     # 2000 rows per core
MT = ROWS // 128 + (1 if ROWS % 128 else 0)  # 16 m-tiles (2000 = 15*128 + 80)

_CACHE = {}


def _build_program():
    nc = bacc.Bacc("TRN2", target_bir_lowering=False, debug=False,
                   num_devices=NCORES)

    xfT_d = nc.declare_dram_parameter("xfT", [DIM, ROWS], F32, isOutput=False)
    wx0_d = nc.declare_dram_parameter("wx0", [2, DIM, G], F32, isOutput=False)
    wh0_d = nc.declare_dram_parameter("wh0", [2, H, G], F32, isOutput=False)
    b0_d = nc.declare_dram_parameter("b0", [2, G], F32, isOutput=False)
    wx1_d = nc.declare_dram_parameter("wx1", [2, 2 * H, G], F32, isOutput=False)
    wh1_d = nc.declare_dram_parameter("wh1", [2, H, G], F32, isOutput=False)
    b1_d = nc.declare_dram_parameter("b1", [2, G], F32, isOutput=False)
    linw_d = nc.declare_dram_parameter("linw", [2 * H, DIM], F32, isOutput=False)
    linb_d = nc.declare_dram_parameter("linb", [DIM], F32, isOutput=False)
    out_d = nc.declare_dram_parameter("out", [ROWS, DIM], F32, isOutput=True)

    xw0_d = nc.dram_tensor("xw0", [2, ROWS, G], F32)
    xw1_d = nc.dram_tensor("xw1", [2, ROWS, G], F32)
    ys0_d = nc.dram_tensor("ys0", [ROWS, 2 * H], F32)
    ys1_d = nc.dram_tensor("ys1", [ROWS, 2 * H], F32)
    ysT_d = nc.dram_tensor("ysT", [2 * H, ROWS], F32R)

    def mrows(m):
        return min(128, ROWS - m * 128)

    with TileContext(nc) as tc:
        with tc.tile_pool(name="const", bufs=1) as constp:
            ident = constp.tile([128, 128], F32)
            make_identity(nc, ident[:])
            ones = constp.tile([1, 128], F32)
            nc.vector.memset(ones[:], 1.0)

            # ---------- batched input projection xw = x @ Wx + b ----------
            def proj_phase(kt, lhsT_load, wx_dram, b_dram, xw_dram):
                """kt: number of 128-K tiles; lhsT_load(m, lt): fill lt tile
                with the (128k x 128m) lhsT tiles for m-tile m."""
                for d in range(2):
                    with tc.tile_pool(name="wxp", bufs=1) as wxp, \
                         tc.tile_pool(name="bbp", bufs=1) as bbp, \
                         tc.tile_pool(name="pp", bufs=4, space="PSUM") as pp, \
                         tc.tile_pool(name="lt", bufs=2) as ltp, \
                         tc.tile_pool(name="xo", bufs=2) as xop:
                        wx_sb = wxp.tile([128, kt, G], F32R)
                        for k in range(kt):
                            nc.sync.dma_start(
                                wx_sb[:, k],
                                wx_dram[d, k * 128:(k + 1) * 128, :].bitcast(F32R))
                        bsb = bbp.tile([1, G], F32)
                        nc.sync.dma_start(bsb[:], b_dram[d][None, :])
                        bb = bbp.tile([128, G], F32)
                        for n in range(6):
                            ns = slice(n * 512, (n + 1) * 512)
                            bps = pp.tile([128, 512], F32, tag="pp")
                            nc.tensor.matmul(bps[:], ones[:], bsb[:, ns],
                                             start=True, stop=True)
                            nc.vector.tensor_copy(bb[:, ns], bps[:])
                        for m in range(MT):
                            mr = mrows(m)
                            lt = ltp.tile([128, kt, 128], F32R, tag="lt")
                            lhsT_load(m, lt)
                            xo = xop.tile([128, G], F32, tag="xo")
                            for n in range(6):
                                ns = slice(n * 512, (n + 1) * 512)
                                ps = pp.tile([mr, 512], F32, tag="pp")
                                for k in range(kt):
                                    nc.tensor.matmul(
                                        ps[:], lt[:, k, :mr], wx_sb[:, k, ns],
                                        start=(k == 0), stop=(k == kt - 1))
                                nc.vector.tensor_tensor(
                                    xo[:mr, ns], ps[:], bb[:mr, ns],
                                    mybir.AluOpType.add)
                            nc.sync.dma_start(
                                xw_dram[d, m * 128:m * 128 + mr, :], xo[:mr])

            def load_from_xfT(m, lt):
                mr = mrows(m)
                for k in range(6):
                    nc.sync.dma_start(
                        lt[:, k, :mr],
                        xfT_d[k * 128:(k + 1) * 128,
                              m * 128:m * 128 + mr].bitcast(F32R))

            def load_from_ysT(m, lt):
                mr = mrows(m)
                for k in range(12):
                    nc.sync.dma_start(
                        lt[:, k, :mr],
                        ysT_d[k * 128:(k + 1) * 128, m * 128:m * 128 + mr])

            # ---------- recurrence (both directions interleaved) ----------
            def recur_phase(wh_dram, xw_dram, ys_dram):
                with tc.tile_pool(name="whp", bufs=1) as whp, \
                     tc.tile_pool(name="st", bufs=1) as stp, \
                     tc.tile_pool(name="pgp", bufs=3) as pgp, \
                     tc.tile_pool(name="gps", bufs=6, space="PSUM") as gpsp, \
                     tc.tile_pool(name="tps", bufs=2, space="PSUM") as tpsp:
                    wh_sb = whp.tile([128, 2, 6, G], F32R)
                    for d in range(2):
                        for k in range(6):
                            nc.sync.dma_start(
                                wh_sb[:, d, k],
                                wh_dram[d, k * 128:(k + 1) * 128, :].bitcast(F32R))
                    h = [stp.tile([SEQ_PC, H], F32, name=f"h{d}") for d in range(2)]
                    c = [stp.tile([SEQ_PC, H], F32, name=f"c{d}") for d in range(2)]
                    tcs = [stp.tile([SEQ_PC, H], F32, name=f"tc{d}") for d in range(2)]
                    tmp = [stp.tile([SEQ_PC, H], F32, name=f"tm{d}") for d in range(2)]
                    hT = [stp.tile([128, 6, SEQ_PC], F32R, name=f"hT{d}")
                          for d in range(2)]
                    for d in range(2):
                        nc.vector.memset(c[d][:], 0.0)

                    xw_r = xw_dram.rearrange("d (s t) g -> d s t g", t=WIDTH)
                    ys_r = ys_dram.rearrange("(s t) g -> s t g", t=WIDTH)

                    for t in range(WIDTH):
                        for d in range(2):
                            tt = t if d == 0 else WIDTH - 1 - t
                            pg = pgp.tile([SEQ_PC, G], F32, tag="pg")
                            nc.sync.dma_start(pg[:], xw_r[d, :, tt, :])
                            if t > 0:
                                for n in range(6):
                                    ns = slice(n * 512, (n + 1) * 512)
                                    ps = gpsp.tile([SEQ_PC, 512], F32, tag="g")
                                    for k in range(6):
                                        nc.tensor.matmul(
                                            ps[:], hT[d][:, k], wh_sb[:, d, k, ns],
                                            start=(k == 0), stop=(k == 5))
                                    nc.vector.tensor_tensor(
                                        pg[:, ns], ps[:], pg[:, ns],
                                        mybir.AluOpType.add)
                            nc.scalar.activation(
                                pg[:, :2304], pg[:, :2304],
                                mybir.ActivationFunctionType.Sigmoid)
                            nc.scalar.activation(
                                pg[:, 2304:], pg[:, 2304:],
                                mybir.ActivationFunctionType.Tanh)
                            # c = f*c + i*g ; h = o*tanh(c)
                            nc.vector.tensor_tensor(
                                tmp[d][:], pg[:, 0:768], pg[:, 2304:3072],
                                mybir.AluOpType.mult)
                            nc.vector.tensor_tensor(
                                c[d][:], c[d][:], pg[:, 768:1536],
                                mybir.AluOpType.mult)
                            nc.vector.tensor_tensor(
                                c[d][:], c[d][:], tmp[d][:], mybir.AluOpType.add)
                            nc.scalar.activation(
                                tcs[d][:], c[d][:],
                                mybir.ActivationFunctionType.Tanh)
                            nc.vector.tensor_tensor(
                                h[d][:], pg[:, 1536:2304], tcs[d][:],
                                mybir.AluOpType.mult)
                            if t < WIDTH - 1:
                                pt = tpsp.tile([128, 6 * SEQ_PC], F32, tag="t")
                                for k in range(6):
                                    nc.tensor.transpose(
                                        pt[:, k * SEQ_PC:(k + 1) * SEQ_PC],
                                        h[d][:, k * 128:(k + 1) * 128],
                                        ident[:SEQ_PC, :SEQ_PC])
                                nc.vector.tensor_copy(
                                    hT[d].rearrange("p k s -> p (k s)"), pt[:])
                            nc.sync.dma_start(
                                ys_r[:, tt, d * H:(d + 1) * H], h[d][:])

            # ---------- transpose ys -> ysT (f32r) ----------
            def transpose_phase(ys_dram):
                with tc.tile_pool(name="ti", bufs=2) as tip, \
                     tc.tile_pool(name="to", bufs=2) as top, \
                     tc.tile_pool(name="tp", bufs=4, space="PSUM") as tpp:
                    for m in range(MT):
                        mr = mrows(m)
                        ti = tip.tile([128, 2 * H], F32, tag="ti")
                        nc.sync.dma_start(
                            ti[:mr], ys_dram[m * 128:m * 128 + mr, :])
                        for k in range(12):
                            ps = tpp.tile([128, 128], F32, tag="tp")
                            nc.tensor.transpose(
                                ps[:, :mr], ti[:mr, k * 128:(k + 1) * 128],
                                ident[:mr, :mr])
                            to = top.tile([128, 128], F32R, tag="to")
                            nc.vector.tensor_copy(to[:, :mr], ps[:, :mr])
                            nc.sync.dma_start(
                                ysT_d[k * 128:(k + 1) * 128,
                                      m * 128:m * 128 + mr], to[:, :mr])

            # ---------- final linear ----------
            def linear_phase():
                with tc.tile_pool(name="lwp", bufs=1) as lwp, \
                     tc.tile_pool(name="lbp", bufs=1) as lbp, \
                     tc.tile_pool(name="lpp", bufs=4, space="PSUM") as lpp, \
                     tc.tile_pool(name="llt", bufs=2) as lltp, \
                     tc.tile_pool(name="lo", bufs=2) as lop:
                    lw = lwp.tile([128, 12, DIM], F32R)
                    for k in range(12):
                        nc.sync.dma_start(
                            lw[:, k],
                            linw_d[k * 128:(k + 1) * 128, :].bitcast(F32R))
                    lbsb = lbp.tile([1, DIM], F32)
                    nc.sync.dma_start(lbsb[:], linb_d[None, :])
                    lbb = lbp.tile([128, DIM], F32)
                    for n in range(2):
                        ns = slice(n * 384, (n + 1) * 384)
                        bps = lpp.tile([128, 384], F32, tag="lp")
                        nc.tensor.matmul(bps[:], ones[:], lbsb[:, ns],
                                         start=True, stop=True)
                        nc.vector.tensor_copy(lbb[:, ns], bps[:])
                    for m in range(MT):
                        mr = mrows(m)
                        lt = lltp.tile([128, 12, 128], F32R, tag="lt")
                        load_from_ysT(m, lt)
                        lo = lop.tile([128, DIM], F32, tag="lo")
                        for n in range(2):
                            ns = slice(n * 384, (n + 1) * 384)
                            ps = lpp.tile([mr, 384], F32, tag="lp")
                            for k in range(12):
                                nc.tensor.matmul(
                                    ps[:], lt[:, k, :mr], lw[:, k, ns],
                                    start=(k == 0), stop=(k == 11))
                            nc.vector.tensor_tensor(
                                lo[:mr, ns], ps[:], lbb[:mr, ns],
                                mybir.AluOpType.add)
                        nc.sync.dma_start(out_d[m * 128:m * 128 + mr, :], lo[:mr])

            proj_phase(6, load_from_xfT, wx0_d, b0_d, xw0_d)
            recur_phase(wh0_d, xw0_d, ys0_d)
            transpose_phase(ys0_d)
            proj_phase(12, load_from_ysT, wx1_d, b1_d, xw1_d)
            recur_phase(wh1_d, xw1_d, ys1_d)
            transpose_phase(ys1_d)
            linear_phase()

    nc.compile()
    return nc


def _reorder_gates(w):
    """[i f g o] -> [i f o g] along last axis (size 4H)."""
    i, f, g, o = np.split(w, 4, axis=-1)
    return np.concatenate([i, f, o, g], axis=-1)


def kernel(x, Wx0f, Wh0f, b0f, Wx0b, Wh0b, b0b,
           Wx1f, Wh1f, b1f, Wx1b, Wh1b, b1b, lin_W, lin_b):
    x = np.asarray(x, dtype=np.float32)
    # frame: (B, C, T) -> (NSEQ, WIDTH, C)
    tgt = (NFR - 1) * STRIDE + WIDTH
    xp = np.zeros((B, DIM, tgt), dtype=np.float32)
    xp[:, :, :T] = x
    frames = np.stack([xp[:, :, i:i + WIDTH]
                       for i in range(0, tgt - WIDTH + 1, STRIDE)], axis=1)
    xf = frames.reshape(NSEQ, DIM, WIDTH).transpose(0, 2, 1)  # (80, 200, 768)

    def prep(wf, wb):
        return np.ascontiguousarray(
            np.stack([_reorder_gates(np.asarray(wf, np.float32)),
                      _reorder_gates(np.asarray(wb, np.float32))]))

    wx0 = prep(Wx0f, Wx0b)
    wh0 = prep(Wh0f, Wh0b)
    b0 = prep(b0f, b0b)
    wx1 = prep(Wx1f, Wx1b)
    wh1 = prep(Wh1f, Wh1b)
    b1 = prep(b1f, b1b)
    linw = np.ascontiguousarray(np.asarray(lin_W, np.float32))
    linb = np.ascontiguousarray(np.asarray(lin_b, np.float32))

    if "nc" not in _CACHE:
        _CACHE["nc"] = _build_program()
    nc = _CACHE["nc"]

    in_maps = []
    for cc in range(NCORES):
        shard = xf[cc * SEQ_PC:(cc + 1) * SEQ_PC]          # (10, 200, 768)
        xfT = np.ascontiguousarray(shard.reshape(ROWS, DIM).T)  # (768, 2000)
        in_maps.append({"xfT": xfT, "wx0": wx0, "wh0": wh0, "b0": b0,
                        "wx1": wx1, "wh1": wh1, "b1": b1,
                        "linw": linw, "linb": linb})

    res = run_bass_kernel_spmd(nc, in_maps, list(range(NCORES)))
    outs = [res.results[cc]["out"].reshape(SEQ_PC, WIDTH, DIM)
            for cc in range(NCORES)]
    y = np.concatenate(outs, axis=0)                        # (80, 200, 768)
    y = y.transpose(0, 2, 1).reshape(B, NFR, DIM, WIDTH)    # (4,20,768,200)

    limit = STRIDE // 2
    parts = [y[:, 0, :, :-limit]]
    for k in range(1, NFR - 1):
        parts.append(y[:, k, :, limit:-limit])
    parts.append(y[:, NFR - 1, :, limit:])
    yc = np.concatenate(parts, axis=-1)[:, :, :T]           # (4, 768, 2000)
    return (yc + x).astype(np.float32)


# revision 4
# speedup vs baseline: 3980.4176x; 3980.4176x over previous
"""Bass/Trainium2 kernel for framed 2-layer BiLSTM (nn_BLSTM).

Data-parallel over the 80 framed sequences: 10 per core on 8 NeuronCores.
Each core runs the full network on its shard: input projections (batched
matmuls), both LSTM directions per layer (interleaved recurrences), final
linear. Host does framing/unframing/skip-add only.
"""
import os
import sys
import numpy as np

sys.path.insert(0, "/opt/trn_rl_repo")

import concourse.bass as bass  # noqa: E402
import concourse.mybir as mybir  # noqa: E402
from concourse import bacc  # noqa: E402
from concourse.tile import TileContext  # noqa: E402
from concourse.masks import make_identity  # noqa: E402
from concourse.bass_utils import run_bass_kernel_spmd  # noqa: E402

F32 = mybir.dt.float32
F32R = mybir.dt.float32r

DIM = 768
H = 768
G = 4 * H            # 3072, gate order reordered to [i, f, o, g]
B, T = 4, 2000
WIDTH, STRIDE = 200, 100
NFR = 20             # frames per batch element
NSEQ = B * NFR       # 80
NCORES = 8
SEQ_PC = NSEQ // NCORES   # 10
ROWS = SEQ_PC * WIDTH     # 2000 rows per core
MT = ROWS // 128 + (1 if ROWS % 128 else 0)  # 16 m-tiles (2000 = 15*128 + 80)

_CACHE = {}


def _build_program():
    nc = bacc.Bacc("TRN2", target_bir_lowering=False, debug=False,
                   num_devices=NCORES)

    xfT_d = nc.declare_dram_parameter("xfT", [DIM, ROWS], F32, isOutput=False)
    wx0_d = nc.declare_dram_parameter("wx0", [2, DIM, G], F32, isOutput=False)
    wh0_d = nc.declare_dram_parameter("wh0", [2, H, G], F32, isOutput=False)
    b0_d = nc.declare_dram_parameter("b0", [2, G], F32, isOutput=False)
    wx1_d = nc.declare_dram_parameter("wx1", [2, 2 * H, G], F32, isOutput=False)
    wh1_d = nc.declare_dram_parameter("wh1", [2, H, G], F32, isOutput=False)
    b1_d = nc.declare_dram_parameter("b1", [2, G], F32, isOutput=False)
    linw_d = nc.declare_dram_parameter("linw", [2 * H, DIM], F32, isOutput=False)
    linb_d = nc.declare_dram_parameter("linb", [DIM], F32, isOutput=False)
    out_d = nc.declare_dram_parameter("out", [ROWS, DIM], F32, isOutput=True)

    xw0_d = nc.dram_tensor("xw0", [2, ROWS, G], F32)
    xw1_d = nc.dram_tensor("xw1", [2, ROWS, G], F32)
    ys0_d = nc.dram_tensor("ys0", [ROWS, 2 * H], F32)
    ys1_d = nc.dram_tensor("ys1", [ROWS, 2 * H], F32)
    ysT_d = nc.dram_tensor("ysT", [2 * H, ROWS], F32R)

    def mrows(m):
        return min(128, ROWS - m * 128)

    with TileContext(nc) as tc:
        with tc.tile_pool(name="const", bufs=1) as constp:
            ident = constp.tile([128, 128], F32)
            make_identity(nc, ident[:])
            ones = constp.tile([1, 128], F32)
            nc.vector.memset(ones[:], 1.0)

            # ---------- batched input projection xw = x @ Wx + b ----------
            def proj_phase(kt, lhsT_load, wx_dram, b_dram, xw_dram):
                """kt: number of 128-K tiles; lhsT_load(m, lt): fill lt tile
                with the (128k x 128m) lhsT tiles for m-tile m."""
                for d in range(2):
                    with tc.tile_pool(name="wxp", bufs=1) as wxp, \
                         tc.tile_pool(name="bbp", bufs=1) as bbp, \
                         tc.tile_pool(name="pp", bufs=4, space="PSUM") as pp, \
                         tc.tile_pool(name="lt", bufs=2) as ltp, \
                         tc.tile_pool(name="xo", bufs=2) as xop:
                        wx_sb = wxp.tile([128, kt, G], F32R)
                        for k in range(kt):
                            nc.sync.dma_start(
                                wx_sb[:, k],
                                wx_dram[d, k * 128:(k + 1) * 128, :].bitcast(F32R))
                        bsb = bbp.tile([1, G], F32)
                        nc.sync.dma_start(bsb[:], b_dram[d][None, :])
                        bb = bbp.tile([128, G], F32)
                        for n in range(6):
                            ns = slice(n * 512, (n + 1) * 512)
                            bps = pp.tile([128, 512], F32, tag="pp")
                            nc.tensor.matmul(bps[:], ones[:], bsb[:, ns],
                                             start=True, stop=True)
                            nc.vector.tensor_copy(bb[:, ns], bps[:])
                        for m in range(MT):
                            mr = mrows(m)
                            lt = ltp.tile([128, kt, 128], F32R, tag="lt")
                            lhsT_load(m, lt)
                            xo = xop.tile([128, G], F32, tag="xo")
                            for n in range(6):
                                ns = slice(n * 512, (n + 1) * 512)
                                ps = pp.tile([mr, 512], F32, tag="pp")
                                for k in range(kt):
                                    nc.tensor.matmul(
                                        ps[:], lt[:, k, :mr], wx_sb[:, k, ns],
                                        start=(k == 0), stop=(k == kt - 1))
                                nc.vector.tensor_tensor(
                                    xo[:mr, ns], ps[:], bb[:mr, ns],
                                    mybir.AluOpType.add)
                            nc.sync.dma_start(
                                xw_dram[d, m * 128:m * 128 + mr, :], xo[:mr])

            def load_from_xfT(m, lt):
                mr = mrows(m)
                for k in range(6):
                    nc.sync.dma_start(
                        lt[:, k, :mr],
                        xfT_d[k * 128:(k + 1) * 128,
                              m * 128:m * 128 + mr].bitcast(F32R))

            def load_from_ysT(m, lt):
                mr = mrows(m)
                for k in range(12):
                    nc.sync.dma_start(
                        lt[:, k, :mr],
                        ysT_d[k * 128:(k + 1) * 128, m * 128:m * 128 + mr])

            # ---------- recurrence (both directions interleaved) ----------
            def recur_phase(wh_dram, xw_dram, ys_dram):
                with tc.tile_pool(name="whp", bufs=1) as whp, \
                     tc.tile_pool(name="st", bufs=1) as stp, \
                     tc.tile_pool(name="pgp", bufs=3) as pgp, \
                     tc.tile_pool(name="gps", bufs=6, space="PSUM") as gpsp, \
                     tc.tile_pool(name="tps", bufs=2, space="PSUM") as tpsp:
                    wh_sb = whp.tile([128, 2, 6, G], F32R)
                    for d in range(2):
                        for k in range(6):
                            nc.sync.dma_start(
                                wh_sb[:, d, k],
                                wh_dram[d, k * 128:(k + 1) * 128, :].bitcast(F32R))
                    h = [stp.tile([SEQ_PC, H], F32, name=f"h{d}") for d in range(2)]
                    c = [stp.tile([SEQ_PC, H], F32, name=f"c{d}") for d in range(2)]
                    tcs = [stp.tile([SEQ_PC, H], F32, name=f"tc{d}") for d in range(2)]
                    tmp = [stp.tile([SEQ_PC, H], F32, name=f"tm{d}") for d in range(2)]
                    hT = [stp.tile([128, 6, SEQ_PC], F32R, name=f"hT{d}")
                          for d in range(2)]
                    for d in range(2):
                        nc.vector.memset(c[d][:], 0.0)

                    xw_r = xw_dram.rearrange("d (s t) g -> d s t g", t=WIDTH)
                    ys_r = ys_dram.rearrange("(s t) g -> s t g", t=WIDTH)

                    for t in range(WIDTH):
                        for d in range(2):
                            tt = t if d == 0 else WIDTH - 1 - t
                            pg = pgp.tile([SEQ_PC, G], F32, tag="pg")
                            nc.sync.dma_start(pg[:], xw_r[d, :, tt, :])
                            if t > 0:
                                for n in range(6):
                                    ns = slice(n * 512, (n + 1) * 512)
                                    ps = gpsp.tile([SEQ_PC, 512], F32, tag="g")
                                    for k in range(6):
                                        nc.tensor.matmul(
                                            ps[:], hT[d][:, k], wh_sb[:, d, k, ns],
                                            start=(k == 0), stop=(k == 5))
                                    nc.vector.tensor_tensor(
                                        pg[:, ns], ps[:], pg[:, ns],
                                        mybir.AluOpType.add)
                            nc.scalar.activation(
                                pg[:, :2304], pg[:, :2304],
                                mybir.ActivationFunctionType.Sigmoid)
                            nc.scalar.activation(
                                pg[:, 2304:], pg[:, 2304:],
                                mybir.ActivationFunctionType.Tanh)
                            # c = f*c + i*g ; h = o*tanh(c)
                            nc.vector.tensor_tensor(
                                tmp[d][:], pg[:, 0:768], pg[:, 2304:3072],
                                mybir.AluOpType.mult)
                            nc.vector.tensor_tensor(
                                c[d][:], c[d][:], pg[:, 768:1536],
                                mybir.AluOpType.mult)
                            nc.vector.tensor_tensor(
                                c[d][:], c[d][:], tmp[d][:], mybir.AluOpType.add)
                            nc.scalar.activation(
                                tcs[d][:], c[d][:],
                                mybir.ActivationFunctionType.Tanh)
                            nc.vector.tensor_tensor(
                                h[d][:], pg[:, 1536:2304], tcs[d][:],
                                mybir.AluOpType.mult)
                            if t < WIDTH - 1:
                                pt = tpsp.tile([128, 6 * SEQ_PC], F32, tag="t")
                                for k in range(6):
                                    nc.tensor.transpose(
                                        pt[:, k * SEQ_PC:(k + 1) * SEQ_PC],
                                        h[d][:, k * 128:(k + 1) * 128],
                                        ident[:SEQ_PC, :SEQ_PC])
                                nc.vector.tensor_copy(
                                    hT[d].rearrange("p k s -> p (k s)"), pt[:])
                            nc.sync.dma_start(
                                ys_r[:, tt, d * H:(d + 1) * H], h[d][:])

            # ---------- transpose ys -> ysT (f32r) ----------
            def transpose_phase(ys_dram):
                with tc.tile_pool(name="ti", bufs=2) as tip, \
                     tc.tile_pool(name="to", bufs=2) as top, \
                     tc.tile_pool(name="tp", bufs=4, space="PSUM") as tpp:
                    for m in range(MT):
                        mr = mrows(m)
                        ti = tip.tile([128, 2 * H], F32, tag="ti")
                        nc.sync.dma_start(
                            ti[:mr], ys_dram[m * 128:m * 128 + mr, :])
                        for k in range(12):
                            ps = tpp.tile([128, 128], F32, tag="tp")
                            nc.tensor.transpose(
                                ps[:, :mr], ti[:mr, k * 128:(k + 1) * 128],
                                ident[:mr, :mr])
                            to = top.tile([128, 128], F32R, tag="to")
                            nc.vector.tensor_copy(to[:, :mr], ps[:, :mr])
                            nc.sync.dma_start(
                                ysT_d[k * 128:(k + 1) * 128,
                                      m * 128:m * 128 + mr], to[:, :mr])

            # ---------- final linear ----------
            def linear_phase():
                with tc.tile_pool(name="lwp", bufs=1) as lwp, \
                     tc.tile_pool(name="lbp", bufs=1) as lbp, \
                     tc.tile_pool(name="lpp", bufs=4, space="PSUM") as lpp, \
                     tc.tile_pool(name="llt", bufs=2) as lltp, \
                     tc.tile_pool(name="lo", bufs=2) as lop:
                    lw = lwp.tile([128, 12, DIM], F32R)
                    for k in range(12):
                        nc.sync.dma_start(
                            lw[:, k],
                            linw_d[k * 128:(k + 1) * 128, :].bitcast(F32R))
                    lbsb = lbp.tile([1, DIM], F32)
                    nc.sync.dma_start(lbsb[:], linb_d[None, :])
                    lbb = lbp.tile([128, DIM], F32)
                    for n in range(2):
                        ns = slice(n * 384, (n + 1) * 384)
                        bps = lpp.tile([128, 384], F32, tag="lp")
                        nc.tensor.matmul(bps[:], ones[:], lbsb[:, ns],
                                         start=True, stop=True)
                        nc.vector.tensor_copy(lbb[:, ns], bps[:])
                    for m in range(MT):
                        mr = mrows(m)
                        lt = lltp.tile([128, 12, 128], F32R, tag="lt")
                        load_from_ysT(m, lt)
                        lo = lop.tile([128, DIM], F32, tag="lo")
                        for n in range(2):
                            ns = slice(n * 384, (n + 1) * 384)
                            ps = lpp.tile([mr, 384], F32, tag="lp")
                            for k in range(12):
                                nc.tensor.matmul(
                                    ps[:], lt[:, k, :mr], lw[:, k, ns],
                                    start=(k == 0), stop=(k == 11))
                            nc.vector.tensor_tensor(
                                lo[:mr, ns], ps[:], lbb[:mr, ns],
                                mybir.AluOpType.add)
                        nc.sync.dma_start(out_d[m * 128:m * 128 + mr, :], lo[:mr])

            proj_phase(6, load_from_xfT, wx0_d, b0_d, xw0_d)
            recur_phase(wh0_d, xw0_d, ys0_d)
            transpose_phase(ys0_d)
            proj_phase(12, load_from_ysT, wx1_d, b1_d, xw1_d)
            recur_phase(wh1_d, xw1_d, ys1_d)
            transpose_phase(ys1_d)
            linear_phase()

    nc.compile()
    return nc


def _reorder_gates(w):
    """[i f g o] -> [i f o g] along last axis (size 4H)."""
    i, f, g, o = np.split(w, 4, axis=-1)
    return np.concatenate([i, f, o, g], axis=-1)


def kernel(x, Wx0f, Wh0f, b0f, Wx0b, Wh0b, b0b,
           Wx1f, Wh1f, b1f, Wx1b, Wh1b, b1b, lin_W, lin_b):
    x = np.asarray(x, dtype=np.float32)
    # frame: (B, C, T) -> (NSEQ, WIDTH, C)
    tgt = (NFR - 1) * STRIDE + WIDTH
    xp = np.zeros((B, DIM, tgt), dtype=np.float32)
    xp[:, :, :T] = x
    frames = np.stack([xp[:, :, i:i + WIDTH]
                       for i in range(0, tgt - WIDTH + 1, STRIDE)], axis=1)
    xf = frames.reshape(NSEQ, DIM, WIDTH).transpose(0, 2, 1)  # (80, 200, 768)

    def prep(wf, wb):
        return np.ascontiguousarray(
            np.stack([_reorder_gates(np.asarray(wf, np.float32)),
                      _reorder_gates(np.asarray(wb, np.float32))]))

    wx0 = prep(Wx0f, Wx0b)
    wh0 = prep(Wh0f, Wh0b)
    b0 = prep(b0f, b0b)
    wx1 = prep(Wx1f, Wx1b)
    wh1 = prep(Wh1f, Wh1b)
    b1 = prep(b1f, b1b)
    linw = np.ascontiguousarray(np.asarray(lin_W, np.float32))
    linb = np.ascontiguousarray(np.asarray(lin_b, np.float32))

    if "nc" not in _CACHE:
        _CACHE["nc"] = _build_program()
    nc = _CACHE["nc"]

    in_maps = []
    for cc in range(NCORES):
        shard = xf[cc * SEQ_PC:(cc + 1) * SEQ_PC]          # (10, 200, 768)
        xfT = np.ascontiguousarray(shard.reshape(ROWS, DIM).T)  # (768, 2000)
        in_maps.append({"xfT": xfT, "wx0": wx0, "wh0": wh0, "b0": b0,
                        "wx1": wx1, "wh1": wh1, "b1": b1,
                        "linw": linw, "linb": linb})
    _CACHE["in_maps"] = in_maps

    trace = os.environ.get("BLSTM_TRACE", "") == "1"
    res = run_bass_kernel_spmd(nc, in_maps, list(range(NCORES)), trace=trace)
    if trace:
        print("profile exec_time_ns:", res.exec_time_ns)
        _CACHE["exec_time_ns"] = res.exec_time_ns
        _CACHE["profile_json"] = res.profile_json
    outs = [res.results[cc]["out"].reshape(SEQ_PC, WIDTH, DIM)
            for cc in range(NCORES)]
    y = np.concatenate(outs, axis=0)                        # (80, 200, 768)
    y = y.transpose(0, 2, 1).reshape(B, NFR, DIM, WIDTH)    # (4,20,768,200)

    limit = STRIDE // 2
    parts = [y[:, 0, :, :-limit]]
    for k in range(1, NFR - 1):
        parts.append(y[:, k, :, limit:-limit])
    parts.append(y[:, NFR - 1, :, limit:])
    yc = np.concatenate(parts, axis=-1)[:, :, :T]           # (4, 768, 2000)
    return (yc + x).astype(np.float32)


# revision 11
# speedup vs baseline: 4115.5552x; 1.0340x over previous
"""Bass/Trainium2 kernel for framed 2-layer BiLSTM (nn_BLSTM).

Data-parallel over the 80 framed sequences: 10 per core on 8 NeuronCores.
Each core runs the full network on its shard: input projections (batched
matmuls), both LSTM directions per layer (interleaved recurrences), final
linear. Host does framing/unframing/skip-add only.
"""
import os
import sys
import numpy as np

sys.path.insert(0, "/opt/trn_rl_repo")

import concourse.bass as bass  # noqa: E402
import concourse.mybir as mybir  # noqa: E402
from concourse import bacc  # noqa: E402
from concourse.tile import TileContext  # noqa: E402
from concourse.masks import make_identity  # noqa: E402
from concourse.bass_utils import run_bass_kernel_spmd  # noqa: E402

F32 = mybir.dt.float32
F32R = mybir.dt.float32r

DIM = 768
H = 768
G = 4 * H            # 3072, gate order reordered to [i, f, o, g]
B, T = 4, 2000
WIDTH, STRIDE = 200, 100
NFR = 20             # frames per batch element
NSEQ = B * NFR       # 80
NCORES = 8
SEQ_PC = NSEQ // NCORES   # 10
ROWS = SEQ_PC * WIDTH     # 2000 rows per core
MT = ROWS // 128 + (1 if ROWS % 128 else 0)  # 16 m-tiles (2000 = 15*128 + 80)

_CACHE = {}


def _build_program():
    nc = bacc.Bacc("TRN2", target_bir_lowering=False, debug=False,
                   num_devices=NCORES)

    xfT_d = nc.declare_dram_parameter("xfT", [DIM, ROWS], F32, isOutput=False)
    wx0_d = nc.declare_dram_parameter("wx0", [2, DIM, G], F32, isOutput=False)
    wh0_d = nc.declare_dram_parameter("wh0", [2, H, G], F32, isOutput=False)
    b0_d = nc.declare_dram_parameter("b0", [2, G], F32, isOutput=False)
    wx1_d = nc.declare_dram_parameter("wx1", [2, 2 * H, G], F32, isOutput=False)
    wh1_d = nc.declare_dram_parameter("wh1", [2, H, G], F32, isOutput=False)
    b1_d = nc.declare_dram_parameter("b1", [2, G], F32, isOutput=False)
    linw_d = nc.declare_dram_parameter("linw", [2 * H, DIM], F32, isOutput=False)
    linb_d = nc.declare_dram_parameter("linb", [DIM], F32, isOutput=False)
    out_d = nc.declare_dram_parameter("out", [ROWS, DIM], F32, isOutput=True)

    xw0_d = nc.dram_tensor("xw0", [2, ROWS, G], F32)
    xw1_d = nc.dram_tensor("xw1", [2, ROWS, G], F32)
    ys0_d = nc.dram_tensor("ys0", [ROWS, 2 * H], F32)
    ys1_d = nc.dram_tensor("ys1", [ROWS, 2 * H], F32)
    ysT_d = nc.dram_tensor("ysT", [2 * H, ROWS], F32R)

    def mrows(m):
        return min(128, ROWS - m * 128)

    with TileContext(nc) as tc:
        with tc.tile_pool(name="const", bufs=1) as constp:
            ident = constp.tile([128, 128], F32)
            make_identity(nc, ident[:])
            ones = constp.tile([1, 128], F32)
            nc.vector.memset(ones[:], 1.0)

            # ---------- batched input projection xw = x @ Wx + b ----------
            def proj_phase(kt, lhsT_load, wx_dram, b_dram, xw_dram):
                """kt: number of 128-K tiles; lhsT_load(m, lt): fill lt tile
                with the (128k x 128m) lhsT tiles for m-tile m."""
                for d in range(2):
                    with tc.tile_pool(name="wxp", bufs=1) as wxp, \
                         tc.tile_pool(name="bbp", bufs=1) as bbp, \
                         tc.tile_pool(name="pp", bufs=4, space="PSUM") as pp, \
                         tc.tile_pool(name="lt", bufs=2) as ltp, \
                         tc.tile_pool(name="xo", bufs=2) as xop:
                        wx_sb = wxp.tile([128, kt, G], F32R)
                        for k in range(kt):
                            nc.sync.dma_start(
                                wx_sb[:, k],
                                wx_dram[d, k * 128:(k + 1) * 128, :].bitcast(F32R))
                        bsb = bbp.tile([1, G], F32)
                        nc.sync.dma_start(bsb[:], b_dram[d][None, :])
                        bb = bbp.tile([128, G], F32)
                        for n in range(6):
                            ns = slice(n * 512, (n + 1) * 512)
                            bps = pp.tile([128, 512], F32, tag="pp")
                            nc.tensor.matmul(bps[:], ones[:], bsb[:, ns],
                                             start=True, stop=True)
                            nc.vector.tensor_copy(bb[:, ns], bps[:])
                        for m in range(MT):
                            mr = mrows(m)
                            lt = ltp.tile([128, kt, 128], F32R, tag="lt")
                            lhsT_load(m, lt)
                            xo = xop.tile([128, G], F32, tag="xo")
                            for n in range(6):
                                ns = slice(n * 512, (n + 1) * 512)
                                ps = pp.tile([mr, 512], F32, tag="pp")
                                for k in range(kt):
                                    nc.tensor.matmul(
                                        ps[:], lt[:, k, :mr], wx_sb[:, k, ns],
                                        start=(k == 0), stop=(k == kt - 1))
                                nc.vector.tensor_tensor(
                                    xo[:mr, ns], ps[:], bb[:mr, ns],
                                    mybir.AluOpType.add)
                            nc.sync.dma_start(
                                xw_dram[d, m * 128:m * 128 + mr, :], xo[:mr])

            def load_from_xfT(m, lt):
                mr = mrows(m)
                for k in range(6):
                    nc.sync.dma_start(
                        lt[:, k, :mr],
                        xfT_d[k * 128:(k + 1) * 128,
                              m * 128:m * 128 + mr].bitcast(F32R))

            def load_from_ysT(m, lt):
                mr = mrows(m)
                for k in range(12):
                    nc.sync.dma_start(
                        lt[:, k, :mr],
                        ysT_d[k * 128:(k + 1) * 128, m * 128:m * 128 + mr])

            # ---------- recurrence (both directions interleaved) ----------
            def recur_phase(wh_dram, xw_dram, ys_dram):
                with tc.tile_pool(name="whp", bufs=1) as whp, \
                     tc.tile_pool(name="st", bufs=1) as stp, \
                     tc.tile_pool(name="pgp", bufs=3) as pgp, \
                     tc.tile_pool(name="gps", bufs=6, space="PSUM") as gpsp, \
                     tc.tile_pool(name="tps", bufs=2, space="PSUM") as tpsp:
                    wh_sb = whp.tile([128, 2, 6, G], F32R)
                    for d in range(2):
                        for k in range(6):
                            nc.sync.dma_start(
                                wh_sb[:, d, k],
                                wh_dram[d, k * 128:(k + 1) * 128, :].bitcast(F32R))
                    h = [stp.tile([SEQ_PC, H], F32, name=f"h{d}") for d in range(2)]
                    c = [stp.tile([SEQ_PC, H], F32, name=f"c{d}") for d in range(2)]
                    tcs = [stp.tile([SEQ_PC, H], F32, name=f"tc{d}") for d in range(2)]
                    tmp = [stp.tile([SEQ_PC, H], F32, name=f"tm{d}") for d in range(2)]
                    hT = [stp.tile([128, 6, SEQ_PC], F32R, name=f"hT{d}")
                          for d in range(2)]
                    for d in range(2):
                        nc.vector.memset(c[d][:], 0.0)

                    xw_r = xw_dram.rearrange("d (s t) g -> d s t g", t=WIDTH)
                    ys_r = ys_dram.rearrange("(s t) g -> s t g", t=WIDTH)

                    abl = os.environ.get("BLSTM_ABLATE", "")
                    for t in range(WIDTH):
                        for d in range(2):
                            tt = t if d == 0 else WIDTH - 1 - t
                            pg = pgp.tile([SEQ_PC, G], F32, tag="pg")
                            nc.sync.dma_start(pg[:], xw_r[d, :, tt, :])
                            if t > 0 and "nomm" not in abl:
                                for n in range(6):
                                    ns = slice(n * 512, (n + 1) * 512)
                                    ps = gpsp.tile([SEQ_PC, 512], F32, tag="g")
                                    for k in range(6):
                                        nc.tensor.matmul(
                                            ps[:], hT[d][:, k], wh_sb[:, d, k, ns],
                                            start=(k == 0), stop=(k == 5))
                                    nc.vector.tensor_tensor(
                                        pg[:, ns], ps[:], pg[:, ns],
                                        mybir.AluOpType.add)
                            # piecewise ACT: each span fires as soon as the
                            # psum-tile adds covering it are done
                            nc.scalar.activation(
                                pg[:, 0:1024], pg[:, 0:1024],
                                mybir.ActivationFunctionType.Sigmoid)
                            nc.scalar.activation(
                                pg[:, 1024:2304], pg[:, 1024:2304],
                                mybir.ActivationFunctionType.Sigmoid)
                            nc.scalar.activation(
                                pg[:, 2304:3072], pg[:, 2304:3072],
                                mybir.ActivationFunctionType.Tanh)
                            # c = f*c + i*g ; h = o*tanh(c)
                            nc.vector.tensor_tensor(
                                tmp[d][:], pg[:, 0:768], pg[:, 2304:3072],
                                mybir.AluOpType.mult)
                            nc.vector.tensor_tensor(
                                c[d][:], c[d][:], pg[:, 768:1536],
                                mybir.AluOpType.mult)
                            nc.vector.tensor_tensor(
                                c[d][:], c[d][:], tmp[d][:], mybir.AluOpType.add)
                            nc.scalar.activation(
                                tcs[d][:], c[d][:],
                                mybir.ActivationFunctionType.Tanh)
                            nc.vector.tensor_tensor(
                                h[d][:], pg[:, 1536:2304], tcs[d][:],
                                mybir.AluOpType.mult)
                            if t < WIDTH - 1 and "notr" not in abl:
                                pt = tpsp.tile([128, 6 * SEQ_PC], F32, tag="t")
                                for k in range(6):
                                    nc.tensor.transpose(
                                        pt[:, k * SEQ_PC:(k + 1) * SEQ_PC],
                                        h[d][:, k * 128:(k + 1) * 128],
                                        ident[:SEQ_PC, :SEQ_PC])
                                nc.vector.tensor_copy(
                                    hT[d].rearrange("p k s -> p (k s)"), pt[:])
                            nc.sync.dma_start(
                                ys_r[:, tt, d * H:(d + 1) * H], h[d][:])

            # ---------- transpose ys -> ysT (f32r) ----------
            def transpose_phase(ys_dram):
                with tc.tile_pool(name="ti", bufs=2) as tip, \
                     tc.tile_pool(name="to", bufs=2) as top, \
                     tc.tile_pool(name="tp", bufs=4, space="PSUM") as tpp:
                    for m in range(MT):
                        mr = mrows(m)
                        ti = tip.tile([128, 2 * H], F32, tag="ti")
                        nc.sync.dma_start(
                            ti[:mr], ys_dram[m * 128:m * 128 + mr, :])
                        for k in range(12):
                            ps = tpp.tile([128, 128], F32, tag="tp")
                            nc.tensor.transpose(
                                ps[:, :mr], ti[:mr, k * 128:(k + 1) * 128],
                                ident[:mr, :mr])
                            to = top.tile([128, 128], F32R, tag="to")
                            nc.vector.tensor_copy(to[:, :mr], ps[:, :mr])
                            nc.sync.dma_start(
                                ysT_d[k * 128:(k + 1) * 128,
                                      m * 128:m * 128 + mr], to[:, :mr])

            # ---------- final linear ----------
            def linear_phase():
                with tc.tile_pool(name="lwp", bufs=1) as lwp, \
                     tc.tile_pool(name="lbp", bufs=1) as lbp, \
                     tc.tile_pool(name="lpp", bufs=4, space="PSUM") as lpp, \
                     tc.tile_pool(name="llt", bufs=2) as lltp, \
                     tc.tile_pool(name="lo", bufs=2) as lop:
                    lw = lwp.tile([128, 12, DIM], F32R)
                    for k in range(12):
                        nc.sync.dma_start(
                            lw[:, k],
                            linw_d[k * 128:(k + 1) * 128, :].bitcast(F32R))
                    lbsb = lbp.tile([1, DIM], F32)
                    nc.sync.dma_start(lbsb[:], linb_d[None, :])
                    lbb = lbp.tile([128, DIM], F32)
                    for n in range(2):
                        ns = slice(n * 384, (n + 1) * 384)
                        bps = lpp.tile([128, 384], F32, tag="lp")
                        nc.tensor.matmul(bps[:], ones[:], lbsb[:, ns],
                                         start=True, stop=True)
                        nc.vector.tensor_copy(lbb[:, ns], bps[:])
                    for m in range(MT):
                        mr = mrows(m)
                        lt = lltp.tile([128, 12, 128], F32R, tag="lt")
                        load_from_ysT(m, lt)
                        lo = lop.tile([128, DIM], F32, tag="lo")
                        for n in range(2):
                            ns = slice(n * 384, (n + 1) * 384)
                            ps = lpp.tile([mr, 384], F32, tag="lp")
                            for k in range(12):
                                nc.tensor.matmul(
                                    ps[:], lt[:, k, :mr], lw[:, k, ns],
                                    start=(k == 0), stop=(k == 11))
                            nc.vector.tensor_tensor(
                                lo[:mr, ns], ps[:], lbb[:mr, ns],
                                mybir.AluOpType.add)
                        nc.sync.dma_start(out_d[m * 128:m * 128 + mr, :], lo[:mr])

            proj_phase(6, load_from_xfT, wx0_d, b0_d, xw0_d)
            recur_phase(wh0_d, xw0_d, ys0_d)
            transpose_phase(ys0_d)
            proj_phase(12, load_from_ysT, wx1_d, b1_d, xw1_d)
            recur_phase(wh1_d, xw1_d, ys1_d)
            transpose_phase(ys1_d)
            linear_phase()

    nc.compile()
    return nc


def _reorder_gates(w):
    """[i f g o] -> [i f o g] along last axis (size 4H)."""
    i, f, g, o = np.split(w, 4, axis=-1)
    return np.concatenate([i, f, o, g], axis=-1)


def kernel(x, Wx0f, Wh0f, b0f, Wx0b, Wh0b, b0b,
           Wx1f, Wh1f, b1f, Wx1b, Wh1b, b1b, lin_W, lin_b):
    x = np.asarray(x, dtype=np.float32)
    # frame: (B, C, T) -> (NSEQ, WIDTH, C)
    tgt = (NFR - 1) * STRIDE + WIDTH
    xp = np.zeros((B, DIM, tgt), dtype=np.float32)
    xp[:, :, :T] = x
    frames = np.stack([xp[:, :, i:i + WIDTH]
                       for i in range(0, tgt - WIDTH + 1, STRIDE)], axis=1)
    xf = frames.reshape(NSEQ, DIM, WIDTH).transpose(0, 2, 1)  # (80, 200, 768)

    def prep(wf, wb):
        return np.ascontiguousarray(
            np.stack([_reorder_gates(np.asarray(wf, np.float32)),
                      _reorder_gates(np.asarray(wb, np.float32))]))

    wx0 = prep(Wx0f, Wx0b)
    wh0 = prep(Wh0f, Wh0b)
    b0 = prep(b0f, b0b)
    wx1 = prep(Wx1f, Wx1b)
    wh1 = prep(Wh1f, Wh1b)
    b1 = prep(b1f, b1b)
    linw = np.ascontiguousarray(np.asarray(lin_W, np.float32))
    linb = np.ascontiguousarray(np.asarray(lin_b, np.float32))

    if "nc" not in _CACHE:
        _CACHE["nc"] = _build_program()
    nc = _CACHE["nc"]

    in_maps = []
    for cc in range(NCORES):
        shard = xf[cc * SEQ_PC:(cc + 1) * SEQ_PC]          # (10, 200, 768)
        xfT = np.ascontiguousarray(shard.reshape(ROWS, DIM).T)  # (768, 2000)
        in_maps.append({"xfT": xfT, "wx0": wx0, "wh0": wh0, "b0": b0,
                        "wx1": wx1, "wh1": wh1, "b1": b1,
                        "linw": linw, "linb": linb})
    _CACHE["in_maps"] = in_maps

    res = run_bass_kernel_spmd(nc, in_maps, list(range(NCORES)))
    outs = [res.results[cc]["out"].reshape(SEQ_PC, WIDTH, DIM)
            for cc in range(NCORES)]
    y = np.concatenate(outs, axis=0)                        # (80, 200, 768)
    y = y.transpose(0, 2, 1).reshape(B, NFR, DIM, WIDTH)    # (4,20,768,200)

    limit = STRIDE // 2
    parts = [y[:, 0, :, :-limit]]
    for k in range(1, NFR - 1):
        parts.append(y[:, k, :, limit:-limit])
    parts.append(y[:, NFR - 1, :, limit:])
    yc = np.concatenate(parts, axis=-1)[:, :, :T]           # (4, 768, 2000)
    return (yc + x).astype(np.float32)
